# revision 1
# baseline (speedup 1.0000x reference)
"""Trainium2 Bass kernel for DenseMLPQMatrixDecoder.

Math: per embedding v, a tiny MLP (8->16->16->16) produces logits for a 4x4
rate matrix Q (zero diag -> exp -> row-normalize off-diag -> diag = -1).
The reference then computes expm(Q*1000) per (v, s) and takes row 0.

Key facts (verified against the reference numerically):
  * site_positions is never used numerically -- the S axis is a pure
    broadcast of the per-v result.
  * The slowest-mixing Q over the input distribution has spectral gap
    ~1.1, so expm(Q*1000) == the stationary distribution pi of Q to well
    below float32 resolution.  pi is computed exactly via the Markov-chain
    tree theorem: pi_i proportional to the (i,i) principal minor of Q
    (all four minors share one sign, so normalization cancels it).

Sharding: V=1024 split as 128 rows per core across 8 cores (pure data
parallel); MLP weights replicated.  Each core computes pi for its 128 v's
([128,4]), replicates along the free dim to [128, S*4], and writes its
contiguous 2MB slice of the output.

Hardware notes (trn2 walrus codegen):
  * Instructions can carry at most ONE sync wait; Bacc.finalize()'s
    legalization (move_matmul_waits_to_ldweights + generate_event_semaphores)
    handles that -- a plain bass.Bass() module fails walrus codegen.
  * All inputs ride ONE dma: weights, biases and the pre-transposed
    embedding shard are host-packed into a single [17, 179] tensor.  The
    last MLP layer uses an augmented ones-row (zero-padded W1 column +
    bias 1.0) so its matmul emits logits v-major, avoiding any on-device
    transpose; the diagonal logit bias of -100 makes exp() produce the
    zeroed Q diagonal for free.
  * The output store is a single repeat-source DMA: SBUF holds one
    [128, 1024] replication of pi and the DMA's stride-0 source AP repeats
    it 4x into the [128, 4096] DRAM slice (4KB inner runs, HBM-bound).
"""

import numpy as np

import concourse.bacc as bacc
import concourse.bass as bass
import concourse.mybir as mybir
import concourse.tile as tile
from concourse.bass_utils import run_bass_kernel_spmd

AF = mybir.ActivationFunctionType
F32 = mybir.dt.float32

V, D, WIDTH, A, S = 1024, 8, 16, 4, 1024
NCORES = 8
VP = V // NCORES          # 128 rows per core
FREE = S * A              # 4096 output elems per row

# Packed input layout [PACK_P, PACK_F] f32 (one DMA -> one semaphore):
#   rows 0:16 cols  0:17   W1 padded with a zero 17th column (so the mm2
#                          output row 16 is 0; relu(0 + bias 1.0) = 1 builds
#                          the ones-row for the augmented-bias last layer)
#   rows 0:17 cols 17:33   Wout with bout as row 16
#   rows 0:16 col  33      b0
#   rows 0:17 col  34      b1 with 1.0 at row 16
#   rows 0:8  cols 35:51   W0
#   rows 0:8  cols 51:179  emb shard, pre-transposed to [D, VP]
PACK_P = WIDTH + 1        # 17
PACK_F = 51 + VP          # 179


def pack_inputs(W0, b0, W1, b1, Wout, bout, emb) -> list[np.ndarray]:
    """Per-core packed input tensors (emb: full [V, D] array)."""
    base = np.zeros((PACK_P, PACK_F), np.float32)
    base[0:WIDTH, 0:16] = W1
    base[0:WIDTH, 17:33] = Wout
    # bout with -100 on diagonal logit positions: exp then yields ~0 on the
    # Q diagonal, removing the on-device diag-zero memset
    bout_aug = np.asarray(bout, np.float32).copy()
    bout_aug[[0, 5, 10, 15]] -= 100.0
    base[WIDTH, 17:33] = bout_aug
    base[0:WIDTH, 33] = b0
    base[0:WIDTH, 34] = b1
    base[WIDTH, 34] = 1.0
    base[0:D, 35:51] = W0
    packs = []
    for c in range(NCORES):
        p = base.copy()
        p[0:D, 51 : 51 + VP] = emb[c * VP : (c + 1) * VP].T
        packs.append(p)
    return packs


def _diag_ap(t):
    """AP selecting the 4 diagonal elements (free offsets 0,5,10,15)."""
    ap = t[:]
    return bass.AP(tensor=ap.tensor, offset=ap.offset, ap=[ap.ap[0], [5, 4]])


def _build_module() -> bass.Bass:
    # Bacc (not plain Bass): its compile()/finalize() pipeline legalizes
    # multi-wait instructions (move_matmul_waits_to_ldweights +
    # generate_event_semaphores) for the TRN2 1-wait-per-instruction limit.
    nc = bacc.Bacc()

    pack_d = nc.declare_dram_parameter("pack", [PACK_P, PACK_F], F32, isOutput=False)
    out_d = nc.declare_dram_parameter("out", [VP, FREE], F32, isOutput=True)

    with tile.TileContext(nc) as tc:
        with (
            tc.tile_pool(name="sb", bufs=1) as sb,
            tc.tile_pool(name="ps", bufs=1, space="PSUM") as ps,
        ):
            # Dummy no-dep activation: pulls the ~1.3us ACT_TABLE_LOAD to the
            # head of the kernel (parallel with the input DMA) instead of the
            # critical path before the first real activation.
            warm = sb.tile([1, 1], F32)
            nc.scalar.activation(warm[:], nc.const_aps.tensor(0.0, (1, 1)), AF.Exp)

            # ---- load everything with ONE dma --------------------------------
            raw = sb.tile([PACK_P, PACK_F], F32)
            nc.sync.dma_start(raw[:], pack_d[:])

            w1_aug = raw[0:WIDTH, 0:17]         # [16,17], col 16 = zeros
            wout_aug = raw[0:PACK_P, 17:33]     # [17,16], row 16 = bout
            w0_sb = raw[0:D, 35:51]
            embT = raw[0:D, 51 : 51 + VP]       # [8,128]
            b0_raw = raw[0:WIDTH, 33:34]
            b1_raw = raw[0:PACK_P, 34:35]

            # ---- MLP in feature-major layout: h_T = W.T @ x_T ----------------
            ps1 = ps.tile([WIDTH, VP], F32)
            nc.tensor.matmul(ps1[:], w0_sb, embT)
            h0 = sb.tile([WIDTH, VP], F32)
            nc.scalar.activation(h0[:], ps1[:], AF.Relu, bias=b0_raw)

            ps2 = ps.tile([PACK_P, VP], F32)    # [17,128]; row 16 = 0 (zero W1 col)
            nc.tensor.matmul(ps2[:], w1_aug, h0[:])
            h1a = sb.tile([PACK_P, VP], F32)    # [17,128]: relu rows + ones row 16
            nc.scalar.activation(h1a[:], ps2[:], AF.Relu, bias=b1_raw)

            # last layer emitted v-major directly: logq[v,k] = h1a.T @ Wout_aug
            ps3 = ps.tile([VP, A * A], F32)
            nc.tensor.matmul(ps3[:], h1a[:], wout_aug)
            e = sb.tile([VP, A * A], F32)       # E = exp(logq + bout)
            nc.scalar.activation(e[:], ps3[:], AF.Exp)

            # ---- build Q: row-normalize, diag=-1 (diag of E is already ~0
            # via the -100 diagonal logit bias) ---------------------------------
            r = sb.tile([VP, A], F32)
            nc.vector.reduce_sum(
                r[:], e[:].rearrange("p (i j) -> p i j", i=A), axis=mybir.AxisListType.X
            )
            rinv = sb.tile([VP, A], F32)
            nc.vector.reciprocal(rinv[:], r[:])
            q = sb.tile([VP, A * A], F32)
            nc.vector.tensor_tensor(
                q[:].rearrange("p (i j) -> p i j", i=A),
                e[:].rearrange("p (i j) -> p i j", i=A),
                rinv[:].unsqueeze(-1).broadcast_to((VP, A, A)),
                op=mybir.AluOpType.mult,
            )
            nc.vector.memset(_diag_ap(q), -1.0)

            # ---- antisymmetric 2x2 minors for row pairs (2,3),(1,3),(1,2) ----
            # M_ab[c,d] = q[a,c]*q[b,d] - q[a,d]*q[b,c]
            # (all on DVE: gpsimd InstTensorTensor with stride-0 broadcast APs
            # crashes the exec unit on HW even though CoreSim accepts it)
            minors = {}
            for (a, b) in [(2, 3), (1, 3), (1, 2)]:
                eng = nc.vector
                outer = sb.tile([VP, A * A], F32, tag=f"outer{a}{b}")
                eng.tensor_tensor(
                    outer[:].rearrange("p (c d) -> p c d", c=A),
                    q[:, 4 * a : 4 * a + 4].unsqueeze(-1).broadcast_to((VP, A, A)),
                    q[:, 4 * b : 4 * b + 4].unsqueeze(1).broadcast_to((VP, A, A)),
                    op=mybir.AluOpType.mult,
                )
                m = sb.tile([VP, A * A], F32, tag=f"m{a}{b}")
                eng.tensor_tensor(
                    m[:].rearrange("p (c d) -> p c d", c=A),
                    outer[:].rearrange("p (c d) -> p c d", c=A),
                    outer[:].rearrange("p (c d) -> p d c", c=A),
                    op=mybir.AluOpType.subtract,
                )
                minors[(a, b)] = m

            # ---- principal 3x3 minors via cyclic Laplace expansion -----------
            # w_i = det of Q with row/col i removed, expanded along row e:
            #   w_i = q[e,c1]*M[c2,c3] + q[e,c2]*M[c3,c1] + q[e,c3]*M[c1,c2]
            dets = [
                (1, (1, 2, 3), minors[(2, 3)]),  # i=0
                (0, (0, 2, 3), minors[(2, 3)]),  # i=1
                (0, (0, 1, 3), minors[(1, 3)]),  # i=2
                (0, (0, 1, 2), minors[(1, 2)]),  # i=3
            ]
            w = sb.tile([VP, A], F32)
            t0 = sb.tile([VP, 1], F32, tag="t0")
            for i, (e_row, (c1, c2, c3), m) in enumerate(dets):
                terms = [(c1, (c2, c3)), (c2, (c3, c1)), (c3, (c1, c2))]
                for k, (qc, (mc, md)) in enumerate(terms):
                    qo = 4 * e_row + qc
                    mo = 4 * mc + md
                    if k == 0:
                        nc.vector.tensor_tensor(
                            t0[:],
                            q[:, qo : qo + 1],
                            m[:, mo : mo + 1],
                            op=mybir.AluOpType.mult,
                        )
                    else:
                        # fused MAC: dst = m[mo]*q[qo] + t0
                        dst_ap = w[:, i : i + 1] if k == 2 else t0[:]
                        nc.vector.scalar_tensor_tensor(
                            dst_ap,
                            m[:, mo : mo + 1],
                            q[:, qo : qo + 1],
                            t0[:],
                            op0=mybir.AluOpType.mult,
                            op1=mybir.AluOpType.add,
                        )

            # ---- normalize + replicate fused: rep[v, r, j] = w[v,j]*winv[v]
            # (one tensor_tensor with double-broadcast sources), then store
            # with a repeat-source DMA (4KB inner runs stay HBM-bound)
            wsum = sb.tile([VP, 1], F32)
            nc.vector.reduce_sum(wsum[:], w[:], axis=mybir.AxisListType.X)
            winv = sb.tile([VP, 1], F32)
            nc.vector.reciprocal(winv[:], wsum[:])
            pi = sb.tile([VP, A], F32)
            nc.vector.tensor_tensor(
                pi[:], w[:], winv[:].broadcast_to((VP, A)), op=mybir.AluOpType.mult
            )
            REPW = 1024
            rep = sb.tile([VP, REPW], F32)
            # single broadcast-source copy: read pi repeatedly with stride-0
            nc.vector.tensor_copy(
                rep[:].rearrange("p (r f) -> p r f", f=A),
                pi[:].unsqueeze(1).broadcast_to((VP, REPW // A, A)),
            )
            nreps = FREE // REPW
            nc.sync.dma_start(
                out_d[:].rearrange("v (r f) -> v r f", r=nreps),
                rep[:].unsqueeze(1).broadcast_to((VP, nreps, REPW)),
            )

    nc.finalize()
    return nc


_NC_CACHE = None


def _get_module():
    global _NC_CACHE
    if _NC_CACHE is None:
        _NC_CACHE = _build_module()
    return _NC_CACHE


def kernel(**inputs) -> np.ndarray:
    emb = np.ascontiguousarray(np.asarray(inputs["embeddings_VxD"], np.float32))
    packs = pack_inputs(
        *[np.asarray(inputs[k], np.float32) for k in ["W0", "b0", "W1", "b1", "Wout", "bout"]],
        emb,
    )
    nc = _get_module()
    in_maps = [{"pack": packs[c]} for c in range(NCORES)]
    res = run_bass_kernel_spmd(nc, in_maps, list(range(NCORES)))
    out = np.concatenate(
        [res.results[c]["out"].reshape(VP, S, A) for c in range(NCORES)], axis=0
    )
    return out



# revision 7
# speedup vs baseline: 1.2760x; 1.2760x over previous
"""Trainium2 Bass kernel for DenseMLPQMatrixDecoder (optimized v2).

Math: per embedding v, a tiny MLP (8->16->16->16) produces logits for a 4x4
rate matrix Q (zero diag -> exp -> row-normalize off-diag -> diag = -1).
The reference computes expm(Q*1000) per (v, s) and takes row 0; at t=1000
that is exactly the stationary distribution pi of Q, broadcast over the
S (site) axis, which is never used numerically.

Pipeline changes vs v1 (35.9us -> target ~25us):
  * All biases folded into the matmuls via an augmented ones-row carried
    through every layer (host-packed); relu runs on DVE (tensor_scalar_max
    from PSUM, ~260ns) instead of ACT (~360ns).
  * The whole MLP runs in fp16 (weights + activations; PSUM accumulation
    stays fp32): single-pass matmuls instead of the walrus fp32 LOW/HIGH
    double-pass -- halves TensorE time.  Verified numerically: total rel
    err 7.1e-4 vs 5.6e-4 for an fp32 MLP (fp16-output-quant dominated).
  * pi is computed from the UNNORMALIZED matrix B = E - diag(rowsum(E)):
    with Q = D^-1 E - I,  pi_i is proportional to z_i * r_i where z is the
    left null vector of B.  z is obtained as the 4D generalized cross
    product of B's columns 1,2,3 (columns of B sum to zero, so any 3 are
    independent and their cross product spans the null space).  This needs
    only 8 wide DVE ops instead of the 42 narrow ops of the adjugate
    expansion:
      - OUT[k,l] = B[:,2+4k]*B[:,3+4l]        (1 TT, broadcast APs)
      - m = OUT - OUT^T                       (1 TT, transposed-view AP)
      - 12 signed terms z_i += B[u]*m[g] grouped into 3 "classes" whose
        (u, g) free-offset patterns are affine in the lane index i=2a+b
        (verified numerically; antisymmetry of m absorbs all signs):
          A: u = 5+8a-4b,  g = 11-10a+3b
          B: u = 9-8a+4b,  g = 13-6a-5b
          C: u = 13-8a-4b, g = 6+6a-3b
        (3 TTs with custom strided APs into P2[v, i, t])
      - z = reduce_X(P2), p = (z*1)*r4 with fused accum_out=psum (STT),
        pinv = 1/psum, pi = p*pinv (tensor_scalar with per-partition AP)
  * Output is written as float16 (1MB/core instead of 2MB; fp16 quant
    error ~5e-4 rel, total ~6e-4) and cast back to f32 on host.

Sharding: V=1024 split 128 rows/core across 8 cores; weights replicated.
"""

import numpy as np

import concourse.bacc as bacc
import concourse.bass as bass
import concourse.mybir as mybir
import concourse.tile as tile
from concourse.bass_utils import run_bass_kernel_spmd

AF = mybir.ActivationFunctionType
F32 = mybir.dt.float32
F16 = mybir.dt.float16
MUL = mybir.AluOpType.mult

V, D, WIDTH, A, S = 1024, 8, 16, 4, 1024
NCORES = 8
VP = V // NCORES          # 128 rows per core
FREE = S * A              # 4096 output elems per row
REPW = 1024               # fp16 elems of the SBUF replication tile (2KB runs)

# Packed input layout [PACK_P, PACK_F] fp16 (one DMA -> one semaphore):
#   rows 0:17 cols  0:17    W1_aug ([[W1,0],[b1,1]])
#   rows 0:17 cols 17:33    Wout_aug ([[Wout],[bout - 100*diagmask]])
#   rows 0:9  cols 33:50    W0_aug ([[W0,0],[b0,1]])
#   rows 0:9  cols 50:178   embT_aug ([emb_shard.T; ones])
PACK_P = WIDTH + 1        # 17
PACK_F = 50 + VP          # 178


def pack_inputs(W0, b0, W1, b1, Wout, bout, emb) -> list[np.ndarray]:
    """Per-core packed input tensors (emb: full [V, D] array)."""
    base = np.zeros((PACK_P, PACK_F), np.float32)
    base[0:WIDTH, 0:WIDTH] = W1
    base[WIDTH, 0:WIDTH] = b1
    base[WIDTH, WIDTH] = 1.0
    base[0:WIDTH, 17:33] = Wout
    bout_aug = np.asarray(bout, np.float32).copy()
    # -100 on diagonal logit positions: exp() then yields ~0 on the E diag
    bout_aug[[0, 5, 10, 15]] -= 100.0
    base[WIDTH, 17:33] = bout_aug
    base[0:D, 33:49] = W0
    base[D, 33:49] = b0
    base[D, 49] = 1.0
    packs = []
    for c in range(NCORES):
        p = base.copy()
        p[0:D, 50 : 50 + VP] = emb[c * VP : (c + 1) * VP].T
        p[D, 50 : 50 + VP] = 1.0
        packs.append(p.astype(np.float16))
    return packs


def _ap(t, off, dims):
    """Custom strided AP into tile t: element offset off, free dims
    [(stride, size), ...] (partition dim preserved)."""
    a = t[:]
    return bass.AP(
        tensor=a.tensor,
        offset=a.offset + off,
        ap=[a.ap[0]] + [[s, n] for (s, n) in dims],
    )


def _build_module() -> bass.Bass:
    nc = bacc.Bacc()

    pack_d = nc.declare_dram_parameter("pack", [PACK_P, PACK_F], F16, isOutput=False)
    out_d = nc.declare_dram_parameter("out", [VP, FREE], F16, isOutput=True)

    with tile.TileContext(nc) as tc:
        with (
            tc.tile_pool(name="sb", bufs=1) as sb,
            tc.tile_pool(name="ps", bufs=1, space="PSUM") as ps,
        ):
            # Dummy no-dep activation: pulls the ~2.7us ACT_TABLE_LOAD to the
            # head of the kernel, overlapping the input DMA.
            warm = sb.tile([1, 1], F32)
            nc.scalar.activation(warm[:], nc.const_aps.tensor(0.0, (1, 1)), AF.Exp)

            # ---- load everything with ONE dma ------------------------------
            raw = sb.tile([PACK_P, PACK_F], F16)
            nc.sync.dma_start(raw[:], pack_d[:])

            w1_aug = raw[0:PACK_P, 0:17]        # [17,17]
            wout_aug = raw[0:PACK_P, 17:33]     # [17,16], row 16 = bout_aug
            w0_aug = raw[0 : D + 1, 33:50]      # [9,17]
            embT = raw[0 : D + 1, 50 : 50 + VP] # [9,128], row 8 = ones

            # ---- MLP, feature-major; ones-row rides through every layer ----
            ps1 = ps.tile([PACK_P, VP], F32)
            nc.tensor.matmul(ps1[:], w0_aug, embT)
            h0 = sb.tile([PACK_P, VP], F16)     # relu rows + ones row 16
            nc.vector.tensor_scalar_max(h0[:], ps1[:], 0.0)

            ps2 = ps.tile([PACK_P, VP], F32)
            nc.tensor.matmul(ps2[:], w1_aug, h0[:])
            h1 = sb.tile([PACK_P, VP], F16)
            nc.vector.tensor_scalar_max(h1[:], ps2[:], 0.0)

            # last layer emitted v-major directly: logits[v,k] = h1.T @ Wout_aug
            ps3 = ps.tile([VP, A * A], F32)
            nc.tensor.matmul(ps3[:], h1[:], wout_aug)
            e = sb.tile([VP, A * A], F32)       # E = exp(logits); diag ~ 0
            nc.scalar.activation(e[:], ps3[:], AF.Exp)

            # ---- B = E - diag(rowsum): write -r4 onto the diagonal ---------
            r4 = sb.tile([VP, A], F32)
            nc.vector.reduce_sum(
                r4[:], e[:].rearrange("p (i j) -> p i j", i=A),
                axis=mybir.AxisListType.X, op=mybir.AluOpType.add,
            )
            nc.vector.tensor_scalar_mul(_ap(e, 0, [(5, 4)]), r4[:], -1.0)

            # ---- z = cross4(col1, col2, col3 of B) -------------------------
            outr = sb.tile([VP, A * A], F32)    # OUT[k,l] = B[2+4k]*B[3+4l]
            nc.vector.tensor_tensor(
                _ap(outr, 0, [(4, 4), (1, 4)]),
                _ap(e, 2, [(4, 4), (0, 4)]),
                _ap(e, 3, [(0, 4), (4, 4)]),
                op=MUL,
            )
            m = sb.tile([VP, A * A], F32)       # m = OUT - OUT^T
            nc.vector.tensor_tensor(
                _ap(m, 0, [(4, 4), (1, 4)]),
                _ap(outr, 0, [(4, 4), (1, 4)]),
                _ap(outr, 0, [(1, 4), (4, 4)]),
                op=mybir.AluOpType.subtract,
            )
            # 12 signed terms in 3 affine classes -> P2[v, i=2a+b, t]
            p2 = sb.tile([VP, A * 3], F32)
            for t, (uo, ua, ub, go, ga, gb) in enumerate([
                (5, 8, -4, 11, -10, 3),    # class A
                (9, -8, 4, 13, -6, -5),    # class B
                (13, -8, -4, 6, 6, -3),    # class C
            ]):
                nc.vector.tensor_tensor(
                    _ap(p2, t, [(6, 2), (3, 2)]),
                    _ap(e, uo, [(ua, 2), (ub, 2)]),
                    _ap(m, go, [(ga, 2), (gb, 2)]),
                    op=MUL,
                )
            z = sb.tile([VP, A], F32)
            nc.vector.reduce_sum(
                z[:], p2[:].rearrange("p (i t) -> p i t", i=A),
                axis=mybir.AxisListType.X, op=mybir.AluOpType.add,
            )

            # ---- pi_i = z_i*r_i / sum_j z_j*r_j  (fused accum_out) ---------
            pvec = sb.tile([VP, A], F32)
            psum = sb.tile([VP, 1], F32)
            nc.vector.scalar_tensor_tensor(
                pvec[:], z[:], 1.0, r4[:], op0=MUL, op1=MUL, accum_out=psum[:]
            )
            pinv = sb.tile([VP, 1], F32)
            nc.vector.reciprocal(pinv[:], psum[:])
            pi16 = sb.tile([VP, A], F16)
            nc.vector.tensor_scalar(
                pi16[:], pvec[:], pinv[:], None, op0=MUL
            )

            # ---- replicate to [VP, REPW] fp16, store with repeat-source DMA
            rep = sb.tile([VP, REPW], F16)
            nc.vector.tensor_copy(
                rep[:].rearrange("p (r f) -> p r f", f=A),
                pi16[:].unsqueeze(1).broadcast_to((VP, REPW // A, A)),
            )
            nreps = FREE // REPW
            nc.sync.dma_start(
                out_d[:].rearrange("v (r f) -> v r f", r=nreps),
                rep[:].unsqueeze(1).broadcast_to((VP, nreps, REPW)),
            )

    nc.finalize()
    return nc


_NC_CACHE = None


def _get_module():
    global _NC_CACHE
    if _NC_CACHE is None:
        _NC_CACHE = _build_module()
    return _NC_CACHE


def kernel(**inputs) -> np.ndarray:
    emb = np.ascontiguousarray(np.asarray(inputs["embeddings_VxD"], np.float32))
    packs = pack_inputs(
        *[np.asarray(inputs[k], np.float32) for k in ["W0", "b0", "W1", "b1", "Wout", "bout"]],
        emb,
    )
    nc = _get_module()
    in_maps = [{"pack": packs[c]} for c in range(NCORES)]
    res = run_bass_kernel_spmd(nc, in_maps, list(range(NCORES)))
    out = np.concatenate(
        [
            res.results[c]["out"].astype(np.float32).reshape(VP, S, A)
            for c in range(NCORES)
        ],
        axis=0,
    )
    return out


# revision 10
# speedup vs baseline: 1.5819x; 1.2397x over previous
"""Trainium2 Bass kernel for DenseMLPQMatrixDecoder (optimized v2).

Math: per embedding v, a tiny MLP (8->16->16->16) produces logits for a 4x4
rate matrix Q (zero diag -> exp -> row-normalize off-diag -> diag = -1).
The reference computes expm(Q*1000) per (v, s) and takes row 0; at t=1000
that is exactly the stationary distribution pi of Q, broadcast over the
S (site) axis, which is never used numerically.

Pipeline changes vs v1 (35.9us -> target ~25us):
  * All biases folded into the matmuls via an augmented ones-row carried
    through every layer (host-packed); relu runs on DVE (tensor_scalar_max
    from PSUM, ~260ns) instead of ACT (~360ns).
  * The whole MLP runs in fp16 (weights + activations; PSUM accumulation
    stays fp32): single-pass matmuls instead of the walrus fp32 LOW/HIGH
    double-pass -- halves TensorE time.  Verified numerically: total rel
    err 7.1e-4 vs 5.6e-4 for an fp32 MLP (fp16-output-quant dominated).
  * pi is computed from the UNNORMALIZED matrix B = E - diag(rowsum(E)):
    with Q = D^-1 E - I,  pi_i is proportional to z_i * r_i where z is the
    left null vector of B.  z is obtained as the 4D generalized cross
    product of B's columns 1,2,3 (columns of B sum to zero, so any 3 are
    independent and their cross product spans the null space).  This needs
    only 8 wide DVE ops instead of the 42 narrow ops of the adjugate
    expansion:
      - OUT[k,l] = B[:,2+4k]*B[:,3+4l]        (1 TT, broadcast APs)
      - m = OUT - OUT^T                       (1 TT, transposed-view AP)
      - 12 signed terms z_i += B[u]*m[g] grouped into 3 "classes" whose
        (u, g) free-offset patterns are affine in the lane index i=2a+b
        (verified numerically; antisymmetry of m absorbs all signs):
          A: u = 5+8a-4b,  g = 11-10a+3b
          B: u = 9-8a+4b,  g = 13-6a-5b
          C: u = 13-8a-4b, g = 6+6a-3b
        (3 TTs with custom strided APs into P2[v, i, t])
      - z = reduce_X(P2), p = (z*1)*r4 with fused accum_out=psum (STT),
        pinv = 1/psum, pi = p*pinv (tensor_scalar with per-partition AP)
  * Output is written as float16 (1MB/core instead of 2MB; fp16 quant
    error ~5e-4 rel, total ~6e-4) and cast back to f32 on host.

Sharding: V=1024 split 128 rows/core across 8 cores; weights replicated.
"""

import numpy as np

import concourse.bacc as bacc
import concourse.bass as bass
import concourse.mybir as mybir
import concourse.tile as tile
from concourse.bass_utils import run_bass_kernel_spmd

AF = mybir.ActivationFunctionType
F32 = mybir.dt.float32
F16 = mybir.dt.float16
MUL = mybir.AluOpType.mult

V, D, WIDTH, A, S = 1024, 8, 16, 4, 1024
NCORES = 8
VP = V // NCORES          # 128 rows per core
FREE = S * A              # 4096 output elems per row
REPW = 1024               # fp16 elems of the SBUF replication tile (2KB runs)

# Packed input layout [PACK_P, PACK_F] fp16 (one DMA -> one semaphore):
#   rows 0:17 cols  0:17    W1_aug ([[W1,0],[b1,1]])
#   rows 0:17 cols 17:33    Wout_aug ([[Wout],[bout - 100*diagmask]])
#   rows 0:9  cols 33:50    W0_aug ([[W0,0],[b0,1]])
#   rows 0:9  cols 50:178   embT_aug ([emb_shard.T; ones])
PACK_P = WIDTH + 1        # 17
PACK_F = 50 + VP          # 178


def pack_inputs(W0, b0, W1, b1, Wout, bout, emb) -> list[np.ndarray]:
    """Per-core packed input tensors (emb: full [V, D] array)."""
    base = np.zeros((PACK_P, PACK_F), np.float32)
    base[0:WIDTH, 0:WIDTH] = W1
    base[WIDTH, 0:WIDTH] = b1
    base[WIDTH, WIDTH] = 1.0
    base[0:WIDTH, 17:33] = Wout
    bout_aug = np.asarray(bout, np.float32).copy()
    # -100 on diagonal logit positions: exp() then yields ~0 on the E diag
    bout_aug[[0, 5, 10, 15]] -= 100.0
    base[WIDTH, 17:33] = bout_aug
    base[0:D, 33:49] = W0
    base[D, 33:49] = b0
    base[D, 49] = 1.0
    packs = []
    for c in range(NCORES):
        p = base.copy()
        p[0:D, 50 : 50 + VP] = emb[c * VP : (c + 1) * VP].T
        p[D, 50 : 50 + VP] = 1.0
        packs.append(p.astype(np.float16))
    return packs


def _ap(t, off, dims):
    """Custom strided AP into tile t: element offset off, free dims
    [(stride, size), ...] (partition dim preserved)."""
    a = t[:]
    return bass.AP(
        tensor=a.tensor,
        offset=a.offset + off,
        ap=[a.ap[0]] + [[s, n] for (s, n) in dims],
    )


def _build_module_raw() -> bass.Bass:
    """Raw Bacc module: manual semaphores (5), back-to-back DVE chain, and a
    fire-and-forget output DMA.  No TileContext.

    Engine streams (program order per engine; 1-wait/inst legalized by
    Bacc.compile):
      Sync : dma_start(pack)->+sIN .................. dma_start(out) [no wait]
      PE   : [sIN>=16] mm1 ->+sPE  [sDVE>=1] mm2 ->+sPE  [sDVE>=2] mm3 ->+sPE
      DVE  : [sPE>=1] relu1 ->+sDVE  [sPE>=2] relu2 ->+sDVE
             [sACT>=1] (pi chain, 12 ops, program order)  rep ->+sDVE
      ACT  : warm-exp          [sPE>=3] exp(e) ->+sACT

    The output DMA has no completion wait in the kernel: its ~4.6us data
    phase drains concurrently with the ~8us NRT postamble (sem resets),
    which otherwise strictly serializes after it.  The NRT dma_rearm at the
    very end of the postamble runs ~2us after the transfer finishes.
    """
    nc = bacc.Bacc()

    pack_d = nc.declare_dram_parameter("pack", [PACK_P, PACK_F], F16, isOutput=False)
    out_d = nc.declare_dram_parameter("out", [VP, FREE], F16, isOutput=True)

    sIN = nc.alloc_semaphore("sIN")
    sPE = nc.alloc_semaphore("sPE")
    sDVE = nc.alloc_semaphore("sDVE")
    sACT = nc.alloc_semaphore("sACT")
    sOUT = nc.alloc_semaphore("sOUT")

    raw = nc.alloc_sbuf_tensor("raw", [PACK_P, PACK_F], F16)
    h0 = nc.alloc_sbuf_tensor("h0", [PACK_P, VP], F16)
    h1 = nc.alloc_sbuf_tensor("h1", [PACK_P, VP], F16)
    e = nc.alloc_sbuf_tensor("e", [VP, A * A], F32)
    r4 = nc.alloc_sbuf_tensor("r4", [VP, A], F32)
    outr = nc.alloc_sbuf_tensor("outr", [VP, A * A], F32)
    m = nc.alloc_sbuf_tensor("m", [VP, A * A], F32)
    p2 = nc.alloc_sbuf_tensor("p2", [VP, A * 3], F32)
    z = nc.alloc_sbuf_tensor("z", [VP, A], F32)
    pvec = nc.alloc_sbuf_tensor("pvec", [VP, A], F32)
    psum_t = nc.alloc_sbuf_tensor("psum", [VP, 1], F32)
    pinv = nc.alloc_sbuf_tensor("pinv", [VP, 1], F32)
    pi16 = nc.alloc_sbuf_tensor("pi16", [VP, A], F16)
    rep = nc.alloc_sbuf_tensor("rep", [VP, REPW], F16)
    warm = nc.alloc_sbuf_tensor("warm", [1, 1], F32)

    ps1 = nc.alloc_psum_tensor("ps1", [PACK_P, VP], F32)
    ps2 = nc.alloc_psum_tensor("ps2", [PACK_P, VP], F32)
    ps3 = nc.alloc_psum_tensor("ps3", [VP, A * A], F32)

    # ---- issue order mirrors the dependency chain -----------------------
    nc.scalar.activation(warm.ap(), nc.const_aps.tensor(0.0, (1, 1)), AF.Exp)
    nc.sync.dma_start(raw.ap(), pack_d[:]).then_inc(sIN, 16)

    w1_aug = raw.ap()[0:PACK_P, 0:17]
    wout_aug = raw.ap()[0:PACK_P, 17:33]
    w0_aug = raw.ap()[0 : D + 1, 33:50]
    embT = raw.ap()[0 : D + 1, 50 : 50 + VP]

    nc.tensor.wait_ge(sIN, 16)
    nc.tensor.matmul(ps1.ap(), w0_aug, embT).then_inc(sPE, 1)
    nc.vector.wait_ge(sPE, 1)
    nc.vector.tensor_scalar_max(h0.ap(), ps1.ap(), 0.0).then_inc(sDVE, 1)
    nc.tensor.wait_ge(sDVE, 1)
    nc.tensor.matmul(ps2.ap(), w1_aug, h0.ap()).then_inc(sPE, 1)
    nc.vector.wait_ge(sPE, 2)
    nc.vector.tensor_scalar_max(h1.ap(), ps2.ap(), 0.0).then_inc(sDVE, 1)
    nc.tensor.wait_ge(sDVE, 2)
    nc.tensor.matmul(ps3.ap(), h1.ap(), wout_aug).then_inc(sPE, 1)
    nc.scalar.wait_ge(sPE, 3)
    nc.scalar.activation(e.ap(), ps3.ap(), AF.Exp).then_inc(sACT, 1)

    # ---- pi chain: one engine, sDVE counter chains the RAW deps ---------
    # (relaxed ordering lets independent same-engine ops overlap -- the
    # three class TTs share one wait threshold and pipeline)
    nc.vector.wait_ge(sACT, 1)
    nc.vector.reduce_sum(
        r4.ap(), e.ap().rearrange("p (i j) -> p i j", i=A),
        axis=mybir.AxisListType.X, op=mybir.AluOpType.add,
    ).then_inc(sDVE, 1)                                        # -> 3
    nc.vector.wait_ge(sDVE, 3)
    nc.vector.tensor_scalar_mul(
        _ap(e, 0, [(5, 4)]), r4.ap(), -1.0
    ).then_inc(sDVE, 1)                                        # -> 4
    nc.vector.wait_ge(sDVE, 4)
    nc.vector.tensor_tensor(
        _ap(outr, 0, [(4, 4), (1, 4)]),
        _ap(e, 2, [(4, 4), (0, 4)]),
        _ap(e, 3, [(0, 4), (4, 4)]),
        op=MUL,
    ).then_inc(sDVE, 1)                                        # -> 5
    nc.vector.wait_ge(sDVE, 5)
    nc.vector.tensor_tensor(
        _ap(m, 0, [(4, 4), (1, 4)]),
        _ap(outr, 0, [(4, 4), (1, 4)]),
        _ap(outr, 0, [(1, 4), (4, 4)]),
        op=mybir.AluOpType.subtract,
    ).then_inc(sDVE, 1)                                        # -> 6
    for t, (uo, ua, ub, go, ga, gb) in enumerate([
        (5, 8, -4, 11, -10, 3),
        (9, -8, 4, 13, -6, -5),
        (13, -8, -4, 6, 6, -3),
    ]):
        nc.vector.wait_ge(sDVE, 6)
        nc.vector.tensor_tensor(
            _ap(p2, t, [(6, 2), (3, 2)]),
            _ap(e, uo, [(ua, 2), (ub, 2)]),
            _ap(m, go, [(ga, 2), (gb, 2)]),
            op=MUL,
        ).then_inc(sDVE, 1)                                    # -> 7,8,9
    nc.vector.wait_ge(sDVE, 9)
    nc.vector.reduce_sum(
        z.ap(), p2.ap().rearrange("p (i t) -> p i t", i=A),
        axis=mybir.AxisListType.X, op=mybir.AluOpType.add,
    ).then_inc(sDVE, 1)                                        # -> 10
    nc.vector.wait_ge(sDVE, 10)
    nc.vector.scalar_tensor_tensor(
        pvec.ap(), z.ap(), 1.0, r4.ap(), op0=MUL, op1=MUL, accum_out=psum_t.ap()
    ).then_inc(sDVE, 1)                                        # -> 11
    nc.vector.wait_ge(sDVE, 11)
    nc.vector.reciprocal(pinv.ap(), psum_t.ap()).then_inc(sDVE, 1)  # -> 12
    nc.vector.wait_ge(sDVE, 12)
    nc.vector.tensor_scalar(
        pi16.ap(), pvec.ap(), pinv.ap(), None, op0=MUL
    ).then_inc(sDVE, 1)                                        # -> 13
    nc.vector.wait_ge(sDVE, 13)
    nc.vector.tensor_copy(
        rep.ap().rearrange("p (r f) -> p r f", f=A),
        pi16.ap().unsqueeze(1).broadcast_to((VP, REPW // A, A)),
    ).then_inc(sDVE, 1)                                        # -> 14

    # ---- fire-and-forget store (drains under the NRT postamble) ---------
    nreps = FREE // REPW
    nc.sync.wait_ge(sDVE, 14)
    nc.sync.dma_start(
        out_d[:].rearrange("v (r f) -> v r f", r=nreps),
        rep.ap().unsqueeze(1).broadcast_to((VP, nreps, REPW)),
    ).then_inc(sOUT, 16)

    nc.finalize()
    return nc


def _build_module() -> bass.Bass:
    nc = bacc.Bacc()

    pack_d = nc.declare_dram_parameter("pack", [PACK_P, PACK_F], F16, isOutput=False)
    out_d = nc.declare_dram_parameter("out", [VP, FREE], F16, isOutput=True)

    with tile.TileContext(nc) as tc:
        with (
            tc.tile_pool(name="sb", bufs=1) as sb,
            tc.tile_pool(name="ps", bufs=1, space="PSUM") as ps,
        ):
            # Dummy no-dep activation: pulls the ~2.7us ACT_TABLE_LOAD to the
            # head of the kernel, overlapping the input DMA.
            warm = sb.tile([1, 1], F32)
            nc.scalar.activation(warm[:], nc.const_aps.tensor(0.0, (1, 1)), AF.Exp)

            # ---- load everything with ONE dma ------------------------------
            raw = sb.tile([PACK_P, PACK_F], F16)
            nc.sync.dma_start(raw[:], pack_d[:])

            w1_aug = raw[0:PACK_P, 0:17]        # [17,17]
            wout_aug = raw[0:PACK_P, 17:33]     # [17,16], row 16 = bout_aug
            w0_aug = raw[0 : D + 1, 33:50]      # [9,17]
            embT = raw[0 : D + 1, 50 : 50 + VP] # [9,128], row 8 = ones

            # ---- MLP, feature-major; ones-row rides through every layer ----
            ps1 = ps.tile([PACK_P, VP], F32)
            nc.tensor.matmul(ps1[:], w0_aug, embT)
            h0 = sb.tile([PACK_P, VP], F16)     # relu rows + ones row 16
            nc.vector.tensor_scalar_max(h0[:], ps1[:], 0.0)

            ps2 = ps.tile([PACK_P, VP], F32)
            nc.tensor.matmul(ps2[:], w1_aug, h0[:])
            h1 = sb.tile([PACK_P, VP], F16)
            nc.vector.tensor_scalar_max(h1[:], ps2[:], 0.0)

            # last layer emitted v-major directly: logits[v,k] = h1.T @ Wout_aug
            ps3 = ps.tile([VP, A * A], F32)
            nc.tensor.matmul(ps3[:], h1[:], wout_aug)
            e = sb.tile([VP, A * A], F32)       # E = exp(logits); diag ~ 0
            nc.scalar.activation(e[:], ps3[:], AF.Exp)

            # ---- B = E - diag(rowsum): write -r4 onto the diagonal ---------
            r4 = sb.tile([VP, A], F32)
            nc.vector.reduce_sum(
                r4[:], e[:].rearrange("p (i j) -> p i j", i=A),
                axis=mybir.AxisListType.X, op=mybir.AluOpType.add,
            )
            nc.vector.tensor_scalar_mul(_ap(e, 0, [(5, 4)]), r4[:], -1.0)

            # ---- z = cross4(col1, col2, col3 of B) -------------------------
            outr = sb.tile([VP, A * A], F32)    # OUT[k,l] = B[2+4k]*B[3+4l]
            nc.vector.tensor_tensor(
                _ap(outr, 0, [(4, 4), (1, 4)]),
                _ap(e, 2, [(4, 4), (0, 4)]),
                _ap(e, 3, [(0, 4), (4, 4)]),
                op=MUL,
            )
            m = sb.tile([VP, A * A], F32)       # m = OUT - OUT^T
            nc.vector.tensor_tensor(
                _ap(m, 0, [(4, 4), (1, 4)]),
                _ap(outr, 0, [(4, 4), (1, 4)]),
                _ap(outr, 0, [(1, 4), (4, 4)]),
                op=mybir.AluOpType.subtract,
            )
            # 12 signed terms in 3 affine classes -> P2[v, i=2a+b, t]
            p2 = sb.tile([VP, A * 3], F32)
            for t, (uo, ua, ub, go, ga, gb) in enumerate([
                (5, 8, -4, 11, -10, 3),    # class A
                (9, -8, 4, 13, -6, -5),    # class B
                (13, -8, -4, 6, 6, -3),    # class C
            ]):
                nc.vector.tensor_tensor(
                    _ap(p2, t, [(6, 2), (3, 2)]),
                    _ap(e, uo, [(ua, 2), (ub, 2)]),
                    _ap(m, go, [(ga, 2), (gb, 2)]),
                    op=MUL,
                )
            z = sb.tile([VP, A], F32)
            nc.vector.reduce_sum(
                z[:], p2[:].rearrange("p (i t) -> p i t", i=A),
                axis=mybir.AxisListType.X, op=mybir.AluOpType.add,
            )

            # ---- pi_i = z_i*r_i / sum_j z_j*r_j  (fused accum_out) ---------
            pvec = sb.tile([VP, A], F32)
            psum = sb.tile([VP, 1], F32)
            nc.vector.scalar_tensor_tensor(
                pvec[:], z[:], 1.0, r4[:], op0=MUL, op1=MUL, accum_out=psum[:]
            )
            pinv = sb.tile([VP, 1], F32)
            nc.vector.reciprocal(pinv[:], psum[:])
            pi16 = sb.tile([VP, A], F16)
            nc.vector.tensor_scalar(
                pi16[:], pvec[:], pinv[:], None, op0=MUL
            )

            # ---- replicate to [VP, REPW] fp16, store with repeat-source DMA
            rep = sb.tile([VP, REPW], F16)
            nc.vector.tensor_copy(
                rep[:].rearrange("p (r f) -> p r f", f=A),
                pi16[:].unsqueeze(1).broadcast_to((VP, REPW // A, A)),
            )
            nreps = FREE // REPW
            nc.sync.dma_start(
                out_d[:].rearrange("v (r f) -> v r f", r=nreps),
                rep[:].unsqueeze(1).broadcast_to((VP, nreps, REPW)),
            )

    nc.finalize()
    return nc


_NC_CACHE = None
USE_RAW = True            # raw module (manual sems, fire-and-forget store)


def _get_module():
    global _NC_CACHE
    if _NC_CACHE is None:
        _NC_CACHE = _build_module_raw() if USE_RAW else _build_module()
    return _NC_CACHE


def kernel(**inputs) -> np.ndarray:
    emb = np.ascontiguousarray(np.asarray(inputs["embeddings_VxD"], np.float32))
    packs = pack_inputs(
        *[np.asarray(inputs[k], np.float32) for k in ["W0", "b0", "W1", "b1", "Wout", "bout"]],
        emb,
    )
    nc = _get_module()
    in_maps = [{"pack": packs[c]} for c in range(NCORES)]
    res = run_bass_kernel_spmd(nc, in_maps, list(range(NCORES)))
    out = np.concatenate(
        [
            res.results[c]["out"].astype(np.float32).reshape(VP, S, A)
            for c in range(NCORES)
        ],
        axis=0,
    )
    return out
